# revision 28
# baseline (speedup 1.0000x reference)
"""Trainium2 Bass kernel for nn_ProbabilisticMap.

Math: for each (batch b, curve-sample t) the reference computes a 2D Gaussian
density over a 64x64 pixel grid:
    out[b,x,y,t] = exp(-0.5 * q) / sqrt((2pi)^2 det)
    q = (p - mean)^T inv(cov) (p - mean),   p = (x, y)
with mean/cov the Bernstein(num_cps[b])-weighted combination of control-point
means/covariances.

Kernel strategy (pure data-parallel over batch, 16 examples per core):
  Small stage (per-(b,t) scalars, ~1k values):
    - Bernstein weights w[(b,k), t] are a row-select from a constant table
      W2[(n,k), t]; the select is a one-hot (over n*8+k) matmul on the PE.
    - mean/cov contractions over k are matmuls against a constant
      block-diagonal selector BD[(b,k), b].
    - 2x2 inverse/det/quadratic-form coefficients on the vector engine.
    - The density is rewritten in pixel-monomial form:
        q' = A x^2 + B xy + C y^2 + D x + E y + F'
      with F' absorbing log((2pi)^2 det) so out = exp(-0.5 q') needs no
      per-column bias.
  Big stage (33.5M outputs):
    - q' = G^T @ Coef : G is the constant [6, 4096] monomial basis of the
      pixel grid, Coef the [6, (b,t)] coefficients -> PE matmul into PSUM.
    - out = exp(-0.5 q') : one scalar-engine activation PSUM->SBUF.
    - DMA [128 pix, 16 b, 64 t] tiles to HBM (t innermost matches layout).
"""

import math

import numpy as np

NCORES = 8
BATCH = 128
B_LOC = BATCH // NCORES  # 16
K = 8
T = 64
W = 64
H = 64
NPIX = W * H  # 4096
NCHUNK = NPIX // 128  # 32
J = 72  # one-hot rows: degree n in 0..8 (num_cps in 1..9), j = n*8 + k

_CACHE = {}


def _constants():
    t = np.linspace(0.0, 1.0, T)
    # Bernstein table: W2[n*8+k, t] = C(n,k) t^k (1-t)^(n-k), 0 for k > n
    w2 = np.zeros((J, T), dtype=np.float64)
    for n in range(J // K):
        for k in range(K):
            if k <= n:
                w2[n * K + k] = math.comb(n, k) * t**k * (1.0 - t) ** (n - k)
    # pixel monomial basis; pix = x*64 + y
    pix = np.arange(NPIX)
    x = (pix // H).astype(np.float64)
    y = (pix % H).astype(np.float64)
    # rows: x^2, xy, y^2, -2x, -2y, 1  (the -2 folds the D/E coefficient
    # scaling into the constant basis, saving two vector ops on device)
    g = np.stack([x * x, x * y, y * y, -2.0 * x, -2.0 * y, np.ones_like(x)])
    # exact bf16 split of G: integer entries <= 3969 fit in Gh + Gl exactly
    import ml_dtypes

    bf16 = ml_dtypes.bfloat16
    gh = g.astype(np.float32).astype(bf16)
    gl = (g.astype(np.float32) - gh.astype(np.float32)).astype(bf16)
    assert np.all(gh.astype(np.float64) + gl.astype(np.float64) == g)
    gstack = np.concatenate([gh, gh, gh, gl, gl, gl], axis=0)  # [36, 4096]
    # pixel-oct permutation: superchunk of 1024 pix -> [par 8, 128 octs],
    # oct i = (x_sub = i//8, y_oct = i%8), pixel = xs*64 + yo*8 + par
    perm = np.zeros(NPIX, dtype=np.int64)
    for sup in range(NPIX // 1024):
        for par in range(8):
            for i in range(128):
                perm[sup * 1024 + par * 128 + i] = (
                    sup * 1024 + (i // 8) * 64 + (i % 8) * 8 + par
                )
    gstack = np.ascontiguousarray(gstack[:, perm])
    # block-diagonal selector: BD[(b,k), b'] = (b == b')
    bd = np.zeros((B_LOC * K, B_LOC), dtype=np.float64)
    for p in range(B_LOC * K):
        bd[p, p // K] = 1.0
    km8 = (np.arange(BATCH) % K - K).astype(np.float64).reshape(1, BATCH)
    iota_j = np.arange(J, dtype=np.float64).reshape(J, 1)
    ones_j = np.ones((1, J), dtype=np.float64)
    # one packed f32 constant table -> single DMA load
    # rows 0..71 cols 0..63: W2 ; all rows cols 64..79: BD ;
    # rows 0..71 col 80: iota_j ; row 0 cols 128..255: (k - 8) pattern
    packed = np.zeros((128, 256), dtype=np.float64)
    packed[0:J, 0:T] = w2
    packed[:, 64:80] = bd
    packed[0:J, 80] = iota_j[:, 0]
    packed[0, 128:256] = km8[0]
    return {
        "GSTACK": gstack,
        "PACKED": packed.astype(np.float32),
    }


def _build_nc():
    import concourse.bacc as bacc
    import concourse.bass as bass
    import concourse.tile as tile
    from concourse import mybir

    f32 = mybir.dt.float32
    alu = mybir.AluOpType
    act = mybir.ActivationFunctionType

    nc = bacc.Bacc(
        "TRN2",
        target_bir_lowering=False,
        debug=False,
        enable_asserts=True,
        num_devices=NCORES,
    )

    inp_d = nc.dram_tensor("inp", [128, 8], f32, kind="ExternalInput")
    nrep_d = nc.dram_tensor("n_rep", [1, 128], f32, kind="ExternalInput")
    out_d = nc.dram_tensor(
        "out", [B_LOC, NCHUNK // 8, 16, 8, 8, T], f32, kind="ExternalOutput"
    )

    consts = _constants()
    g_d = nc.inline_tensor(consts["GSTACK"], name="gstack_tbl")
    packed_d = nc.inline_tensor(consts["PACKED"], name="packed_tbl")

    log2pi2 = float(2.0 * math.log(2.0 * math.pi))

    with tile.TileContext(nc) as tc:
        _dma_rr = [0]

        def dma(out, in_):
            eng = (nc.sync, nc.scalar, nc.gpsimd)[_dma_rr[0] % 3]
            _dma_rr[0] += 1
            eng.dma_start(out=out, in_=in_)

        with (
            tc.tile_pool(name="const", bufs=1) as cpool,
            tc.tile_pool(name="small", bufs=1) as sm,
            tc.tile_pool(name="bigout", bufs=4) as big,
        ):
            sps = tc.alloc_tile_pool(name="spsum", bufs=1, space="PSUM")
            # ---- constant / input loads (3 DMAs) ----
            bf = mybir.dt.bfloat16
            nrep_s = cpool.tile([1, 128], f32, tag="nrep")
            nc.sync.dma_start(out=nrep_s, in_=nrep_d.ap())
            packed_s = cpool.tile([128, 256], f32, tag="packed")
            nc.scalar.dma_start(out=packed_s, in_=packed_d.ap())
            inp_s = cpool.tile([128, 8], f32, tag="inp")
            nc.gpsimd.dma_start(out=inp_s, in_=inp_d.ap())
            g_s = cpool.tile([36, NPIX], bf, tag="g")
            nc.sync.dma_start(out=g_s, in_=g_d.ap())
            w2_s = packed_s[0:J, 0:T]
            bd_s = packed_s[:, 64:80]
            iotj_s = packed_s[0:J, 80:81]
            km8_s = packed_s[0:1, 128:256]
            cxy_s = inp_s[:, 0:2]
            cov_s = inp_s[:, 2:6]

            # ---- small stage: per-(b,t) coefficients ----
            # j = (num_cps-1)*8 + k = 8*n + (k-8)
            jr = sm.tile([1, 128], f32, tag="jr")
            nc.vector.tensor_scalar(
                out=jr, in0=nrep_s, scalar1=8.0, scalar2=None, op0=alu.mult
            )
            nc.vector.tensor_tensor(out=jr, in0=jr, in1=km8_s, op=alu.add)

            # broadcast j across J partitions via K=1 matmul with a ones column
            ones_col = cpool.tile([1, J], f32, tag="onesj")
            nc.vector.memset(ones_col, 1.0)
            jb_ps = sps.tile([J, 128], f32, tag="jb")
            nc.tensor.matmul(jb_ps, lhsT=ones_col, rhs=jr, start=True, stop=True)

            onehot = sm.tile([J, 128], f32, tag="onehot")
            nc.vector.tensor_scalar(
                out=onehot, in0=jb_ps, scalar1=iotj_s, scalar2=None, op0=alu.is_equal
            )

            # Bernstein weights w[(b,k), t] and w^2 (ACT square, off the
            # DVE critical chain; weighted muls read PSUM directly)
            w_ps = sps.tile([128, T], f32, tag="wps")
            nc.tensor.matmul(w_ps, lhsT=onehot, rhs=w2_s, start=True, stop=True)
            w2q_s = sm.tile([128, T], f32, tag="w2q")
            nc.scalar.square(w2q_s, w_ps)

            # all 5 weighted rows into one tile, ONE contraction matmul
            wm_all = sm.tile([128, 5, T], f32, tag="wm_all")
            for i, (src, scale_tile, col) in enumerate(
                [
                    (w_ps, cxy_s, 0),
                    (w_ps, cxy_s, 1),
                    (w2q_s, cov_s, 0),
                    (w2q_s, cov_s, 1),
                    (w2q_s, cov_s, 3),
                ]
            ):
                nc.vector.tensor_scalar(
                    out=wm_all[:, i, :],
                    in0=src,
                    scalar1=scale_tile[:, col : col + 1],
                    scalar2=None,
                    op0=alu.mult,
                )
            mm_ps = sps.tile([B_LOC, 5, T], f32, tag="ps", bufs=1)
            nc.tensor.matmul(
                mm_ps,
                lhsT=bd_s,
                rhs=wm_all[:].rearrange("p c t -> p (c t)"),
                start=True,
                stop=True,
            )
            mm = sm.tile([B_LOC, 5, T], f32, tag="mm")
            nc.vector.tensor_copy(mm, mm_ps)
            mx = mm[:, 0, :]
            my = mm[:, 1, :]
            ca = mm[:, 2, :]
            cb = mm[:, 3, :]
            cc = mm[:, 4, :]

            def vt(tag):
                return sm.tile([B_LOC, T], f32, tag=tag, name=tag)

            mul, add, sub = alu.mult, alu.add, alu.subtract

            def tt(out, a, b, op):
                nc.vector.tensor_tensor(out=out, in0=a, in1=b, op=op)

            # coefficients A,B,C,D,E,F as slices of one [16 b, 6, 64 t] tile
            cfs = sm.tile([B_LOC, 6, T], f32, tag="cfs")
            A_ = cfs[:, 0, :]
            B_ = cfs[:, 1, :]
            C_ = cfs[:, 2, :]
            D_ = cfs[:, 3, :]
            E_ = cfs[:, 4, :]
            F_ = cfs[:, 5, :]

            det = vt("det")
            tmp = vt("tmp")
            tt(det, ca, cc, mul)
            tt(tmp, cb, cb, mul)
            tt(det, det, tmp, sub)
            rdet = vt("rdet")
            nc.vector.reciprocal(rdet, det)

            i01n = vt("i01n")  # +b/det (true inv offdiag is the negative)
            tt(A_, cc, rdet, mul)  # i00
            tt(i01n, cb, rdet, mul)
            tt(C_, ca, rdet, mul)  # i11
            nc.vector.tensor_scalar(
                out=B_, in0=i01n, scalar1=-2.0, scalar2=None, op0=mul
            )

            d1 = vt("d1")
            d2 = vt("d2")
            tt(d1, A_, mx, mul)  # i00*Mx
            tt(d2, i01n, my, mul)  # (b/det)*My
            tt(D_, d1, d2, sub)  # pairs with the -2x basis row

            e1 = vt("e1")
            e2 = vt("e2")
            tt(e1, C_, my, mul)
            tt(e2, i01n, mx, mul)
            tt(E_, e1, e2, sub)  # pairs with the -2y basis row

            # F' = i00 Mx^2 + i11 My^2 - 2 (b/det) Mx My + ln(det) + 2 ln(2pi)
            f1 = vt("f1")
            f2 = vt("f2")
            f3 = vt("f3")
            tt(f1, d1, mx, mul)
            tt(f2, e1, my, mul)
            tt(f3, d2, mx, mul)
            nc.vector.tensor_scalar(
                out=f3, in0=f3, scalar1=-2.0, scalar2=None, op0=mul
            )
            tt(F_, f1, f2, add)
            tt(F_, F_, f3, add)
            ld = vt("ld")
            nc.scalar.activation(ld, det, func=act.Ln)
            # preload the Exp table on ACT while the prologue continues, so
            # the first real EXP doesn't eat an ACT_TABLE_LOAD
            dummy = sm.tile([1, 1], f32, tag="dummy")
            nc.scalar.activation(dummy, det[0:1, 0:1], func=act.Exp, scale=-0.5)
            tt(F_, F_, ld, add)  # 2*ln(2pi) is folded into the EXP bias

            # exact 3-way bf16 split of the coefficients; with the 2-way G
            # split the stacked K=36 bf16 matmul reproduces the fp32 product
            # exactly (all cross terms kept, fp32 PSUM accumulation).
            cfl = cfs[:].rearrange("p c t -> p (c t)")  # [16, 384]
            sp_h = sm.tile([B_LOC, 6, T], bf, tag="sp_h")
            nc.vector.tensor_copy(sp_h[:].rearrange("p c t -> p (c t)"), cfl)
            h_f = sm.tile([B_LOC, 6 * T], f32, tag="h_f")
            nc.vector.tensor_copy(h_f, sp_h[:].rearrange("p c t -> p (c t)"))
            r1 = sm.tile([B_LOC, 6 * T], f32, tag="r1")
            nc.vector.tensor_tensor(out=r1, in0=cfl, in1=h_f, op=alu.subtract)
            sp_m = sm.tile([B_LOC, 6, T], bf, tag="sp_m")
            nc.vector.tensor_copy(sp_m[:].rearrange("p c t -> p (c t)"), r1)
            m_f = sm.tile([B_LOC, 6 * T], f32, tag="m_f")
            nc.vector.tensor_copy(m_f, sp_m[:].rearrange("p c t -> p (c t)"))
            r2 = sm.tile([B_LOC, 6 * T], f32, tag="r2")
            nc.vector.tensor_tensor(out=r2, in0=r1, in1=m_f, op=alu.subtract)
            sp_l = sm.tile([B_LOC, 6, T], bf, tag="sp_l")
            nc.vector.tensor_copy(sp_l[:].rearrange("p c t -> p (c t)"), r2)

            # collapse [16 b, 64 t] -> cstack row [1, 1024] per (split, coef)
            nbt = B_LOC * T
            cstack = cpool.tile([36, nbt], bf, tag="cstack")
            for si, src in enumerate([sp_h, sp_m, sp_l]):
                for c in range(6):
                    dma(cstack[6 * si + c : 6 * si + c + 1, :], src[:, c, :])
            nc.sync.dma_start(out=cstack[18:27, :], in_=cstack[0:9, :])
            nc.scalar.dma_start(out=cstack[27:36, :], in_=cstack[9:18, :])

            # small-stage PSUM no longer needed; free its banks for bpsum
            sps.release()

            # ---- big stage ----
            # superchunk = 512 pixels as 128 y-quads; the 4 parities get
            # separate matmuls (G columns pre-permuted) and the EXPs
            # interleave them in SBUF so each partition's (par, t) run is
            # 1 KiB contiguous in HBM.
            out_ap = out_d.ap()
            bias_2pi = cpool.tile([128, 1], f32, tag="bias2pi")
            nc.vector.memset(bias_2pi, float(-math.log(2.0 * math.pi)))
            with tc.tile_pool(name="bpsum", bufs=4, space="PSUM") as bps:
                for sup in range(NCHUNK // 8):
                    o = big.tile([128, B_LOC, 8, T], f32, tag="o")
                    for par in range(8):
                        q_ps = bps.tile([128, B_LOC, T], f32, tag="q")
                        for bg in range(2):
                            nc.tensor.matmul(
                                q_ps[:, bg * 8 : (bg + 1) * 8, :],
                                lhsT=g_s[
                                    :,
                                    sup * 1024
                                    + par * 128 : sup * 1024
                                    + par * 128
                                    + 128,
                                ],
                                rhs=cstack[:, bg * 512 : (bg + 1) * 512],
                                start=True,
                                stop=True,
                            )
                        nc.scalar.activation(
                            o[:, :, par, :],
                            q_ps,
                            func=act.Exp,
                            scale=-0.5,
                            bias=bias_2pi,
                        )
                    dst0 = out_ap[0:8, sup].rearrange("b xs yo par t -> xs yo b par t")
                    nc.sync.dma_start(out=dst0, in_=o[:, 0:8])
                    dst1 = out_ap[8:16, sup].rearrange(
                        "b xs yo par t -> xs yo b par t"
                    )
                    nc.scalar.dma_start(out=dst1, in_=o[:, 8:16])

    nc.compile()
    return nc


def _get_nc():
    if "nc" not in _CACHE:
        _CACHE["nc"] = _build_nc()
    return _CACHE["nc"]


def make_in_maps(cp_means, num_cps, cp_covariances):
    cp_means = np.asarray(cp_means, dtype=np.float32)
    cp_covariances = np.asarray(cp_covariances, dtype=np.float32)
    num_cps = np.asarray(num_cps)
    in_maps = []
    for c in range(NCORES):
        bsl = slice(c * B_LOC, (c + 1) * B_LOC)
        cxy = cp_means[:, bsl, :].transpose(1, 0, 2).reshape(128, 2)
        cab = cp_covariances[:, bsl].transpose(1, 0, 2, 3).reshape(128, 4)
        inp = np.zeros((128, 8), dtype=np.float32)
        inp[:, 0:2] = cxy
        inp[:, 2:6] = cab
        nrep = np.repeat(num_cps[bsl].astype(np.float32), K).reshape(1, 128)
        in_maps.append(
            {
                "inp": inp,
                "n_rep": np.ascontiguousarray(nrep),
            }
        )
    return in_maps


def kernel(cp_means, num_cps, cp_covariances):
    from concourse.bass_utils import run_bass_kernel_spmd

    nc = _get_nc()
    in_maps = make_in_maps(cp_means, num_cps, cp_covariances)
    res = run_bass_kernel_spmd(nc, in_maps, list(range(NCORES))).results
    out = np.concatenate(
        [res[i]["out"].reshape(B_LOC, W, H, T) for i in range(NCORES)], axis=0
    )
    return np.ascontiguousarray(out, dtype=np.float32)


# revision 31
# speedup vs baseline: 1.0060x; 1.0060x over previous
"""Trainium2 Bass kernel for nn_ProbabilisticMap.

Math: for each (batch b, curve-sample t) the reference computes a 2D Gaussian
density over a 64x64 pixel grid:
    out[b,x,y,t] = exp(-0.5 * q) / sqrt((2pi)^2 det)
    q = (p - mean)^T inv(cov) (p - mean),   p = (x, y)
with mean/cov the Bernstein(num_cps[b])-weighted combination of control-point
means/covariances.

Kernel strategy (pure data-parallel over batch, 16 examples per core):
  Small stage (per-(b,t) scalars, ~1k values):
    - Bernstein weights w[(b,k), t] are a row-select from a constant table
      W2[(n,k), t]; the select is a one-hot (over n*8+k) matmul on the PE.
    - mean/cov contractions over k are matmuls against a constant
      block-diagonal selector BD[(b,k), b].
    - 2x2 inverse/det/quadratic-form coefficients on the vector engine.
    - The density is rewritten in pixel-monomial form:
        q' = A x^2 + B xy + C y^2 + D x + E y + F'
      with F' absorbing log((2pi)^2 det) so out = exp(-0.5 q') needs no
      per-column bias.
  Big stage (33.5M outputs):
    - q' = G^T @ Coef : G is the constant [6, 4096] monomial basis of the
      pixel grid, Coef the [6, (b,t)] coefficients -> PE matmul into PSUM.
    - out = exp(-0.5 q') : one scalar-engine activation PSUM->SBUF.
    - DMA [128 pix, 16 b, 64 t] tiles to HBM (t innermost matches layout).
"""

import math

import numpy as np

NCORES = 8
BATCH = 128
B_LOC = BATCH // NCORES  # 16
K = 8
T = 64
W = 64
H = 64
NPIX = W * H  # 4096
NCHUNK = NPIX // 128  # 32
J = 72  # one-hot rows: degree n in 0..8 (num_cps in 1..9), j = n*8 + k

_CACHE = {}


def _constants():
    t = np.linspace(0.0, 1.0, T)
    # Bernstein table: W2[n*8+k, t] = C(n,k) t^k (1-t)^(n-k), 0 for k > n
    w2 = np.zeros((J, T), dtype=np.float64)
    for n in range(J // K):
        for k in range(K):
            if k <= n:
                w2[n * K + k] = math.comb(n, k) * t**k * (1.0 - t) ** (n - k)
    # pixel monomial basis; pix = x*64 + y
    pix = np.arange(NPIX)
    x = (pix // H).astype(np.float64)
    y = (pix % H).astype(np.float64)
    # rows: x^2, xy, y^2, -2x, -2y, 1  (the -2 folds the D/E coefficient
    # scaling into the constant basis, saving two vector ops on device)
    g = np.stack([x * x, x * y, y * y, -2.0 * x, -2.0 * y, np.ones_like(x)])
    # exact bf16 split of G: integer entries <= 3969 fit in Gh + Gl exactly
    import ml_dtypes

    bf16 = ml_dtypes.bfloat16
    gh = g.astype(np.float32).astype(bf16)
    gl = (g.astype(np.float32) - gh.astype(np.float32)).astype(bf16)
    assert np.all(gh.astype(np.float64) + gl.astype(np.float64) == g)
    gstack = np.concatenate([gh, gh, gh, gl, gl, gl], axis=0)  # [36, 4096]
    # pixel-quad permutation: superchunk of 512 pix -> [par 4, 128 quads],
    # quad i = (x_sub = i//16, y_quad = i%16), pixel = xs*64 + yq*4 + par
    perm = np.zeros(NPIX, dtype=np.int64)
    for sup in range(NPIX // 512):
        for par in range(4):
            for i in range(128):
                perm[sup * 512 + par * 128 + i] = (
                    sup * 512 + (i // 16) * 64 + (i % 16) * 4 + par
                )
    gstack = np.ascontiguousarray(gstack[:, perm])
    # block-diagonal selector: BD[(b,k), b'] = (b == b')
    bd = np.zeros((B_LOC * K, B_LOC), dtype=np.float64)
    for p in range(B_LOC * K):
        bd[p, p // K] = 1.0
    km8 = (np.arange(BATCH) % K - K).astype(np.float64).reshape(1, BATCH)
    iota_j = np.arange(J, dtype=np.float64).reshape(J, 1)
    ones_j = np.ones((1, J), dtype=np.float64)
    # one packed f32 constant table -> single DMA load
    # rows 0..71 cols 0..63: W2 ; all rows cols 64..79: BD ;
    # rows 0..71 col 80: iota_j ; row 0 cols 128..255: (k - 8) pattern
    packed = np.zeros((128, 256), dtype=np.float64)
    packed[0:J, 0:T] = w2
    packed[:, 64:80] = bd
    packed[0:J, 80] = iota_j[:, 0]
    packed[0, 128:256] = km8[0]
    return {
        "GSTACK": gstack,
        "PACKED": packed.astype(np.float32),
    }


def _build_nc():
    import concourse.bacc as bacc
    import concourse.bass as bass
    import concourse.tile as tile
    from concourse import mybir

    f32 = mybir.dt.float32
    alu = mybir.AluOpType
    act = mybir.ActivationFunctionType

    nc = bacc.Bacc(
        "TRN2",
        target_bir_lowering=False,
        debug=False,
        enable_asserts=True,
        num_devices=NCORES,
    )

    inp_d = nc.dram_tensor("inp", [128, 8], f32, kind="ExternalInput")
    nrep_d = nc.dram_tensor("n_rep", [1, 128], f32, kind="ExternalInput")
    out_d = nc.dram_tensor(
        "out", [B_LOC, NCHUNK // 4, 8, 16, 4, T], f32, kind="ExternalOutput"
    )

    consts = _constants()
    g_d = nc.inline_tensor(consts["GSTACK"], name="gstack_tbl")
    packed_d = nc.inline_tensor(consts["PACKED"], name="packed_tbl")

    log2pi2 = float(2.0 * math.log(2.0 * math.pi))

    with tile.TileContext(nc) as tc:
        _dma_rr = [0]

        def dma(out, in_):
            eng = (nc.sync, nc.scalar, nc.gpsimd)[_dma_rr[0] % 3]
            _dma_rr[0] += 1
            eng.dma_start(out=out, in_=in_)

        with (
            tc.tile_pool(name="const", bufs=1) as cpool,
            tc.tile_pool(name="small", bufs=1) as sm,
            tc.tile_pool(name="bigout", bufs=6) as big,
        ):
            sps = tc.alloc_tile_pool(name="spsum", bufs=1, space="PSUM")
            # ---- constant / input loads (3 DMAs) ----
            bf = mybir.dt.bfloat16
            nrep_s = cpool.tile([1, 128], f32, tag="nrep")
            nc.sync.dma_start(out=nrep_s, in_=nrep_d.ap())
            packed_s = cpool.tile([128, 256], f32, tag="packed")
            nc.scalar.dma_start(out=packed_s, in_=packed_d.ap())
            inp_s = cpool.tile([128, 8], f32, tag="inp")
            nc.gpsimd.dma_start(out=inp_s, in_=inp_d.ap())
            g_s = cpool.tile([36, NPIX], bf, tag="g")
            nc.sync.dma_start(out=g_s, in_=g_d.ap())
            w2_s = packed_s[0:J, 0:T]
            bd_s = packed_s[:, 64:80]
            iotj_s = packed_s[0:J, 80:81]
            km8_s = packed_s[0:1, 128:256]
            cxy_s = inp_s[:, 0:2]
            cov_s = inp_s[:, 2:6]

            # ---- small stage: per-(b,t) coefficients ----
            # j = (num_cps-1)*8 + k = 8*n + (k-8)
            jr = sm.tile([1, 128], f32, tag="jr")
            nc.vector.tensor_scalar(
                out=jr, in0=nrep_s, scalar1=8.0, scalar2=None, op0=alu.mult
            )
            nc.vector.tensor_tensor(out=jr, in0=jr, in1=km8_s, op=alu.add)

            # broadcast j across J partitions via K=1 matmul with a ones column
            ones_col = cpool.tile([1, J], f32, tag="onesj")
            nc.vector.memset(ones_col, 1.0)
            jb_ps = sps.tile([J, 128], f32, tag="jb")
            nc.tensor.matmul(jb_ps, lhsT=ones_col, rhs=jr, start=True, stop=True)

            onehot = sm.tile([J, 128], f32, tag="onehot")
            nc.vector.tensor_scalar(
                out=onehot, in0=jb_ps, scalar1=iotj_s, scalar2=None, op0=alu.is_equal
            )

            # Bernstein weights w[(b,k), t] and w^2 (ACT square, off the
            # DVE critical chain; weighted muls read PSUM directly)
            w_ps = sps.tile([128, T], f32, tag="wps")
            nc.tensor.matmul(w_ps, lhsT=onehot, rhs=w2_s, start=True, stop=True)
            w2q_s = sm.tile([128, T], f32, tag="w2q")
            nc.scalar.square(w2q_s, w_ps)

            # all 5 weighted rows into one tile, ONE contraction matmul
            wm_all = sm.tile([128, 5, T], f32, tag="wm_all")
            for i, (src, scale_tile, col) in enumerate(
                [
                    (w_ps, cxy_s, 0),
                    (w_ps, cxy_s, 1),
                    (w2q_s, cov_s, 0),
                    (w2q_s, cov_s, 1),
                    (w2q_s, cov_s, 3),
                ]
            ):
                nc.vector.tensor_scalar(
                    out=wm_all[:, i, :],
                    in0=src,
                    scalar1=scale_tile[:, col : col + 1],
                    scalar2=None,
                    op0=alu.mult,
                )
            mm_ps = sps.tile([B_LOC, 5, T], f32, tag="ps", bufs=1)
            nc.tensor.matmul(
                mm_ps,
                lhsT=bd_s,
                rhs=wm_all[:].rearrange("p c t -> p (c t)"),
                start=True,
                stop=True,
            )
            mm = sm.tile([B_LOC, 5, T], f32, tag="mm")
            nc.vector.tensor_copy(mm, mm_ps)
            mx = mm[:, 0, :]
            my = mm[:, 1, :]
            ca = mm[:, 2, :]
            cb = mm[:, 3, :]
            cc = mm[:, 4, :]

            def vt(tag):
                return sm.tile([B_LOC, T], f32, tag=tag, name=tag)

            mul, add, sub = alu.mult, alu.add, alu.subtract

            def tt(out, a, b, op):
                nc.vector.tensor_tensor(out=out, in0=a, in1=b, op=op)

            # coefficients A,B,C,D,E,F as slices of one [16 b, 6, 64 t] tile
            cfs = sm.tile([B_LOC, 6, T], f32, tag="cfs")
            A_ = cfs[:, 0, :]
            B_ = cfs[:, 1, :]
            C_ = cfs[:, 2, :]
            D_ = cfs[:, 3, :]
            E_ = cfs[:, 4, :]
            F_ = cfs[:, 5, :]

            det = vt("det")
            tmp = vt("tmp")
            tt(det, ca, cc, mul)
            tt(tmp, cb, cb, mul)
            tt(det, det, tmp, sub)
            rdet = vt("rdet")
            nc.vector.reciprocal(rdet, det)

            i01n = vt("i01n")  # +b/det (true inv offdiag is the negative)
            tt(A_, cc, rdet, mul)  # i00
            tt(i01n, cb, rdet, mul)
            tt(C_, ca, rdet, mul)  # i11
            nc.vector.tensor_scalar(
                out=B_, in0=i01n, scalar1=-2.0, scalar2=None, op0=mul
            )

            d1 = vt("d1")
            d2 = vt("d2")
            tt(d1, A_, mx, mul)  # i00*Mx
            tt(d2, i01n, my, mul)  # (b/det)*My
            tt(D_, d1, d2, sub)  # pairs with the -2x basis row

            e1 = vt("e1")
            e2 = vt("e2")
            nc.gpsimd.tensor_tensor(out=e1, in0=C_, in1=my, op=mul)
            nc.gpsimd.tensor_tensor(out=e2, in0=i01n, in1=mx, op=mul)
            nc.gpsimd.tensor_tensor(out=E_, in0=e1, in1=e2, op=sub)

            # F' = i00 Mx^2 + i11 My^2 - 2 (b/det) Mx My + ln(det) + 2 ln(2pi)
            f1 = vt("f1")
            f2 = vt("f2")
            f3 = vt("f3")
            tt(f1, d1, mx, mul)
            nc.gpsimd.tensor_tensor(out=f2, in0=e1, in1=my, op=mul)
            tt(f3, d2, mx, mul)
            nc.vector.tensor_scalar(
                out=f3, in0=f3, scalar1=-2.0, scalar2=None, op0=mul
            )
            tt(F_, f1, f2, add)
            tt(F_, F_, f3, add)
            ld = vt("ld")
            nc.scalar.activation(ld, det, func=act.Ln)
            # preload the Exp table on ACT while the prologue continues, so
            # the first real EXP doesn't eat an ACT_TABLE_LOAD
            dummy = sm.tile([1, 1], f32, tag="dummy")
            nc.scalar.activation(dummy, det[0:1, 0:1], func=act.Exp, scale=-0.5)
            tt(F_, F_, ld, add)  # 2*ln(2pi) is folded into the EXP bias

            # exact 3-way bf16 split of the coefficients; with the 2-way G
            # split the stacked K=36 bf16 matmul reproduces the fp32 product
            # exactly (all cross terms kept, fp32 PSUM accumulation).
            cfl = cfs[:].rearrange("p c t -> p (c t)")  # [16, 384]
            sp_h = sm.tile([B_LOC, 6, T], bf, tag="sp_h")
            nc.vector.tensor_copy(sp_h[:].rearrange("p c t -> p (c t)"), cfl)
            h_f = sm.tile([B_LOC, 6 * T], f32, tag="h_f")
            nc.vector.tensor_copy(h_f, sp_h[:].rearrange("p c t -> p (c t)"))
            r1 = sm.tile([B_LOC, 6 * T], f32, tag="r1")
            nc.vector.tensor_tensor(out=r1, in0=cfl, in1=h_f, op=alu.subtract)
            sp_m = sm.tile([B_LOC, 6, T], bf, tag="sp_m")
            nc.vector.tensor_copy(sp_m[:].rearrange("p c t -> p (c t)"), r1)
            m_f = sm.tile([B_LOC, 6 * T], f32, tag="m_f")
            nc.vector.tensor_copy(m_f, sp_m[:].rearrange("p c t -> p (c t)"))
            r2 = sm.tile([B_LOC, 6 * T], f32, tag="r2")
            nc.vector.tensor_tensor(out=r2, in0=r1, in1=m_f, op=alu.subtract)
            sp_l = sm.tile([B_LOC, 6, T], bf, tag="sp_l")
            nc.vector.tensor_copy(sp_l[:].rearrange("p c t -> p (c t)"), r2)

            # collapse [16 b, 64 t] -> cstack row [1, 1024] per (split, coef)
            nbt = B_LOC * T
            cstack = cpool.tile([36, nbt], bf, tag="cstack")
            for si, src in enumerate([sp_h, sp_m, sp_l]):
                for c in range(6):
                    dma(cstack[6 * si + c : 6 * si + c + 1, :], src[:, c, :])
            nc.sync.dma_start(out=cstack[18:23, :], in_=cstack[0:5, :])
            nc.scalar.dma_start(out=cstack[23:27, :], in_=cstack[5:9, :])
            nc.gpsimd.dma_start(out=cstack[27:32, :], in_=cstack[9:14, :])
            nc.sync.dma_start(out=cstack[32:36, :], in_=cstack[14:18, :])

            # small-stage PSUM no longer needed; free its banks for bpsum
            sps.release()

            # ---- big stage ----
            # superchunk = 512 pixels as 128 y-quads; the 4 parities get
            # separate matmuls (G columns pre-permuted) and the EXPs
            # interleave them in SBUF so each partition's (par, t) run is
            # 1 KiB contiguous in HBM.
            out_ap = out_d.ap()
            bias_2pi = cpool.tile([128, 1], f32, tag="bias2pi")
            nc.vector.memset(bias_2pi, float(-math.log(2.0 * math.pi)))
            with tc.tile_pool(name="bpsum", bufs=4, space="PSUM") as bps:
                for sup in range(NCHUNK // 4):
                    o = big.tile([128, B_LOC, 4, T], f32, tag="o")
                    for par in range(4):
                        q_ps = bps.tile([128, B_LOC, T], f32, tag="q")
                        for bg in range(2):
                            nc.tensor.matmul(
                                q_ps[:, bg * 8 : (bg + 1) * 8, :],
                                lhsT=g_s[
                                    :,
                                    sup * 512
                                    + par * 128 : sup * 512
                                    + par * 128
                                    + 128,
                                ],
                                rhs=cstack[:, bg * 512 : (bg + 1) * 512],
                                start=True,
                                stop=True,
                            )
                        nc.scalar.activation(
                            o[:, :, par, :],
                            q_ps,
                            func=act.Exp,
                            scale=-0.5,
                            bias=bias_2pi,
                        )
                    dst = out_ap[:, sup].rearrange("b xs yq par t -> xs yq b par t")
                    eng = nc.sync if sup % 2 == 0 else nc.scalar
                    eng.dma_start(out=dst, in_=o[:])

    nc.compile()
    return nc


def _get_nc():
    if "nc" not in _CACHE:
        _CACHE["nc"] = _build_nc()
    return _CACHE["nc"]


def make_in_maps(cp_means, num_cps, cp_covariances):
    cp_means = np.asarray(cp_means, dtype=np.float32)
    cp_covariances = np.asarray(cp_covariances, dtype=np.float32)
    num_cps = np.asarray(num_cps)
    in_maps = []
    for c in range(NCORES):
        bsl = slice(c * B_LOC, (c + 1) * B_LOC)
        cxy = cp_means[:, bsl, :].transpose(1, 0, 2).reshape(128, 2)
        cab = cp_covariances[:, bsl].transpose(1, 0, 2, 3).reshape(128, 4)
        inp = np.zeros((128, 8), dtype=np.float32)
        inp[:, 0:2] = cxy
        inp[:, 2:6] = cab
        nrep = np.repeat(num_cps[bsl].astype(np.float32), K).reshape(1, 128)
        in_maps.append(
            {
                "inp": inp,
                "n_rep": np.ascontiguousarray(nrep),
            }
        )
    return in_maps


def kernel(cp_means, num_cps, cp_covariances):
    from concourse.bass_utils import run_bass_kernel_spmd

    nc = _get_nc()
    in_maps = make_in_maps(cp_means, num_cps, cp_covariances)
    res = run_bass_kernel_spmd(nc, in_maps, list(range(NCORES))).results
    out = np.concatenate(
        [res[i]["out"].reshape(B_LOC, W, H, T) for i in range(NCORES)], axis=0
    )
    return np.ascontiguousarray(out, dtype=np.float32)
